# revision 8
# baseline (speedup 1.0000x reference)
"""Trainium2 Bass kernel for EnhancedHAN (4-relation HANConv + semantic
attention + residual head), SPMD across 8 NeuronCores.

Strategy:
- Host: one global node permutation per node type (snake deal by degree) so
  each 128-node dst tile has a near-uniform edge count; edges sorted by
  (dst tile, src subtable); all gather indices precomputed as int16.
- Device (one Tile program, identical on all 8 cores):
  Phase A: each core projects its 1/8 slice of nodes -> fp16 features +
    fp32 attention logits packed into 768B table rows (alpha logits come
    from the same matmul via [Wp | Wp@A | Wr] column concat).
  AllGather the two node-type feature tables.
  Phase B: per dst tile, batched dma_gather of source rows and per-edge dst
    alpha; edge softmax via exp(alpha) weights (per-segment max subtraction
    is unnecessary: alpha is bounded and softmax is shift-invariant);
    aggregation via one-hot matmuls accumulating [num | den] in PSUM.
  Semantic attention: per-tile tanh/colsum partials, 4KB AllReduce,
    metapath softmax on device.
  Phase C: combine metapaths, final projections + residual; host unpermutes
    the per-core output slices.
"""

import numpy as np

NCORES = 8
P = 128
IN = 256
C = 256
H = 4
Dh = 64
OUT = 128
NEG = 0.2

ND, NS, E = 40000, 30000, 300000

ROW = 384          # feature-table row: 256 fp16 hd + 16 fp32 alpha + pad = 768B
AROW = 64          # alpha-table row stride in f32 elems (256B)
MAXSUB = 20480     # drug subtable split (int16 gather idx limit)
MAXCH_G = 20       # max chunks per feat gather batch (SBUF bound)
MAXN_IDX_SP = 1024

_COMPILED = None
LAST_RESULT = None


# ---------------------------------------------------------------------------
# host-side preprocessing
# ---------------------------------------------------------------------------

def _snake_perm(n_real, n_pad, weight):
    total = n_real + n_pad
    ntiles = total // P
    order = np.argsort(-weight, kind="stable").astype(np.int64)
    ids = np.concatenate([order, np.full(n_pad, -1, np.int64)])
    mat = ids.reshape(P, ntiles).copy()
    mat[1::2] = mat[1::2, ::-1]
    return mat.T.reshape(-1)


def _wrap16(flat):
    a = np.asarray(flat, np.int16).reshape(-1, 16).T
    return np.tile(a, (8, 1))


class Prep:
    pass


def _prep(inp, cfg):
    ndp, nsp = cfg["NDp"], cfg["NSp"]
    nd, ns = cfg["ND"], cfg["NS"]
    TD, TS = cfg["TD"], cfg["TS"]
    RD, RS = TD * P, TS * P
    sub_d = cfg["SUBD"]

    p = Prep()
    rel_defs = [
        ("rev", inp["ei_rev_src"], inp["ei_rev_dst"], "dis", "drug"),
        ("dd", inp["ei_dd_src"], inp["ei_dd_dst"], "drug", "drug"),
        ("treats", inp["ei_treats_src"], inp["ei_treats_dst"], "drug", "dis"),
        ("ss", inp["ei_ss_src"], inp["ei_ss_dst"], "dis", "dis"),
    ]
    deg_rev = np.bincount(inp["ei_rev_dst"], minlength=nd)
    deg_dd = np.bincount(inp["ei_dd_dst"], minlength=nd)
    deg_tr = np.bincount(inp["ei_treats_dst"], minlength=ns)
    deg_ss = np.bincount(inp["ei_ss_dst"], minlength=ns)

    perm_d = _snake_perm(nd, ndp - nd, deg_rev + deg_dd)
    perm_s = _snake_perm(ns, nsp - ns, deg_tr + deg_ss)
    pos_d = np.empty(nd, np.int64)
    pos_d[perm_d[perm_d >= 0]] = np.nonzero(perm_d >= 0)[0]
    pos_s = np.empty(ns, np.int64)
    pos_s[perm_s[perm_s >= 0]] = np.nonzero(perm_s >= 0)[0]
    p.perm_d, p.perm_s = perm_d, perm_s

    pos_of = {"drug": pos_d, "dis": pos_s}
    ntiles_of = {"drug": TD, "dis": TS}
    groups_of = {"drug": [(0, sub_d), (sub_d, ndp - sub_d)],
                 "dis": [(0, nsp)]}

    edge_data = []
    for rname, esrc, edst, stype, dtype_ in rel_defs:
        spos = pos_of[stype][np.asarray(esrc)]
        dpos = pos_of[dtype_][np.asarray(edst)]
        T = ntiles_of[dtype_]
        gtile = dpos // P
        slot = dpos % P
        groups = groups_of[stype]
        if len(groups) == 2:
            grp = (spos >= groups[1][0]).astype(np.int64)
        else:
            grp = np.zeros(len(spos), np.int64)
        order = np.lexsort((grp, gtile))
        spos, slot_s, gtile_s, grp_s = (spos[order], slot[order],
                                        gtile[order], grp[order])
        aoff = 0 if dtype_ == "drug" else RD
        tile_in_core = gtile_s % T
        a_idx = aoff + tile_in_core * P + slot_s

        percore = []
        for c in range(NCORES):
            tiles = []
            lo = c * T
            for i in range(T):
                m = gtile_s == (lo + i)
                per_g = []
                for gi, (gb, gn) in enumerate(groups):
                    mg = m & (grp_s == gi)
                    per_g.append((spos[mg] - gb, a_idx[mg], slot_s[mg]))
                tiles.append(per_g)
            percore.append(tiles)
        edge_data.append((rname, stype, dtype_, groups, percore))

    sched = []
    for rname, stype, dtype_, groups, percore in edge_data:
        T = ntiles_of[dtype_]
        CHG = np.zeros((T, len(groups)), np.int64)
        for c in range(NCORES):
            for i in range(T):
                for g in range(len(groups)):
                    n = len(percore[c][i][g][0])
                    CHG[i, g] = max(CHG[i, g], (n + P - 1) // P)
        for i in range(T):
            if CHG[i].sum() == 0:
                CHG[i, 0] = 1
        batches = []
        cur = []
        s = np.zeros(len(groups), np.int64)
        for i in range(T):
            if cur and np.any(s + CHG[i] > MAXCH_G):
                batches.append(cur)
                cur, s = [], np.zeros(len(groups), np.int64)
            cur.append(i)
            s = s + CHG[i]
        if cur:
            batches.append(cur)
        sched.append({"name": rname, "stype": stype, "dtype": dtype_,
                      "groups": groups, "CHG": CHG, "batches": batches})
    p.sched = sched

    idx_cols = [0, 0, 0]
    ds_cols = 0
    for rs in sched:
        ngrp = len(rs["groups"])
        for b in rs["batches"]:
            nb = [int(rs["CHG"][b, g].sum()) for g in range(ngrp)]
            for g in range(ngrp):
                idx_cols[g] += nb[g] * 8
            idx_cols[2] += sum(nb) * 8
            ds_cols += sum(nb)
    p.idx_cols, p.ds_cols = idx_cols, ds_cols

    idx0 = [np.zeros((P, max(idx_cols[0], 8)), np.int16) for _ in range(NCORES)]
    idx1 = [np.zeros((P, max(idx_cols[1], 8)), np.int16) for _ in range(NCORES)]
    idxa = [np.zeros((P, max(idx_cols[2], 8)), np.int16) for _ in range(NCORES)]
    dslot = [np.full((P, ds_cols), 240.0, np.float32) for _ in range(NCORES)]

    off = [0, 0, 0]
    doff = 0
    name_to_k = {rs["name"]: k for k, rs in enumerate(sched)}
    for rs in sched:
        k = name_to_k[rs["name"]]
        _, _, _, groups, percore = edge_data[k]
        ngrp = len(groups)
        CHG = rs["CHG"]
        for b in rs["batches"]:
            nb = [int(CHG[b, g].sum()) for g in range(ngrp)]
            tot = sum(nb)
            for c in range(NCORES):
                fflat = [np.zeros(nb[g] * P, np.int64) for g in range(ngrp)]
                aflat = np.zeros(tot * P, np.int64)
                dflat = np.full(tot * P, 240.0, np.float32)
                go = [0] * ngrp
                to = 0
                for i in b:
                    for g in range(ngrp):
                        fi, ai, sl = percore[c][i][g]
                        n = len(fi)
                        cap = int(CHG[i, g]) * P
                        fflat[g][go[g]:go[g] + n] = fi
                        aflat[to:to + n] = ai
                        dflat[to:to + n] = sl
                        go[g] += cap
                        to += cap
                for g in range(ngrp):
                    if nb[g]:
                        blk = _wrap16(fflat[g])
                        tgt = idx0 if g == 0 else idx1
                        tgt[c][:, off[g]:off[g] + nb[g] * 8] = blk
                if tot:
                    idxa[c][:, off[2]:off[2] + tot * 8] = _wrap16(aflat)
                    dslot[c][:, doff:doff + tot] = dflat.reshape(tot, P).T
            for g in range(ngrp):
                off[g] += nb[g] * 8
            off[2] += tot * 8
            doff += tot
    p.idx0, p.idx1, p.idxa, p.dslot = idx0, idx1, idxa, dslot

    xd = np.asarray(inp["x_drug"], np.float32)
    xs = np.asarray(inp["x_disease"], np.float32)
    p.xpd, p.xps = [], []
    for c in range(NCORES):
        seld = perm_d[c * RD:(c + 1) * RD]
        sels = perm_s[c * RS:(c + 1) * RS]
        xdp = np.zeros((RD, IN), np.float32)
        xsp = np.zeros((RS, IN), np.float32)
        md, ms = seld >= 0, sels >= 0
        xdp[md] = xd[seld[md]]
        xsp[ms] = xs[sels[ms]]
        p.xpd.append(xdp)
        p.xps.append(xsp)

    def blockdiag(a):
        a = np.asarray(a, np.float64)
        m = np.zeros((C, H), np.float64)
        for h in range(H):
            m[h * Dh:(h + 1) * Dh, h] = a[h]
        return m

    A_drug = np.concatenate([
        blockdiag(inp["a_src_treats"]), blockdiag(inp["a_dst_rev"]),
        blockdiag(inp["a_src_dd"]), blockdiag(inp["a_dst_dd"])], axis=1)
    A_dis = np.concatenate([
        blockdiag(inp["a_dst_treats"]), blockdiag(inp["a_src_rev"]),
        blockdiag(inp["a_src_ss"]), blockdiag(inp["a_dst_ss"])], axis=1)
    p.src_off = {"rev": 4, "dd": 8, "treats": 0, "ss": 8}
    p.dst_off = {"rev": 4, "dd": 12, "treats": 0, "ss": 12}

    Wr = np.asarray(inp["Wr"], np.float64)
    br = np.asarray(inp["br"], np.float64)
    p.w = {}
    for ty, Wp, bp, A in (("d", inp["Wp_drug"], inp["bp_drug"], A_drug),
                          ("s", inp["Wp_dis"], inp["bp_dis"], A_dis)):
        Wp = np.asarray(Wp, np.float64)
        bp = np.asarray(bp, np.float64)
        p.w["wp16" + ty] = Wp.astype(np.float16)
        p.w["bp16" + ty] = bp.reshape(1, C).astype(np.float16)
        wf = np.concatenate([Wp @ A, Wr], axis=1)
        bf = np.concatenate([bp @ A, br]).reshape(1, 16 + OUT)
        p.w["wf32" + ty] = wf.astype(np.float32)
        p.w["bf32" + ty] = bf.astype(np.float32)
    p.w["wk16"] = np.asarray(inp["Wk"], np.float32).astype(np.float16)
    p.w["bk16"] = np.asarray(inp["bk"], np.float32).reshape(1, C).astype(np.float16)
    p.w["q32"] = np.asarray(inp["q"], np.float32).reshape(1, C)
    p.w["wd16"] = np.asarray(inp["Wd"], np.float32).astype(np.float16)
    p.w["bd16"] = np.asarray(inp["bd"], np.float32).reshape(1, OUT).astype(np.float16)
    p.w["wdis16"] = np.asarray(inp["Wdis"], np.float32).astype(np.float16)
    p.w["bdis16"] = np.asarray(inp["bdis"], np.float32).reshape(1, OUT).astype(np.float16)
    return p


# ---------------------------------------------------------------------------
# device program
# ---------------------------------------------------------------------------

def _dma_gather_raw(nc, out_ap, in_ap, idxs_ap, num_idxs, elem_size,
                    elem_step):
    """bass.dma_gather without the elem_size%256 restriction (read size is
    free; only the row stride is 256B-quantized in the ISA)."""
    import concourse.mybir as mybir
    g = nc.gpsimd
    dts = mybir.dt.size(in_ap.dtype)
    stride_bytes = elem_step * dts
    assert stride_bytes % 256 == 0 and stride_bytes // 256 < 256
    assert in_ap.ap[0][0] == elem_step and in_ap.ap[-1][1] == elem_size
    _in = g.lower_ap_dma(in_ap, for_custom_bir_dma=True)
    _idx = g.lower_ap(idxs_ap)
    _out = g.lower_ap(out_ap)
    return g.add_instruction(mybir.InstDMAGatherAnt(
        name=nc.get_next_instruction_name(),
        ins=[*_in, _idx, g.lower_val_access(g.to_reg(num_idxs))],
        outs=[_out], transpose=False, num_idxs=num_idxs, elem_size=elem_size,
        stride_bytes_256=stride_bytes // 256, gen_mode=0,
        single_packet=num_idxs <= MAXN_IDX_SP, queue_num=0))


def _bc(ap, n):
    import concourse.bass as bass
    return bass.AP(ap.tensor, ap.offset, [list(x) for x in ap.ap] + [[0, n]])


def _mid_bc(ap, n):
    import concourse.bass as bass
    new = [list(ap.ap[0])] + [[0, n]] + [list(x) for x in ap.ap[1:]]
    return bass.AP(ap.tensor, ap.offset, new)


def _build(cfg):
    import concourse.bacc as bacc
    import concourse.mybir as mybir
    import concourse.tile as tile
    from concourse.masks import make_identity

    f16 = mybir.dt.float16
    f32 = mybir.dt.float32
    i16 = mybir.dt.int16
    i32 = mybir.dt.int32
    AF = mybir.ActivationFunctionType
    ALU = mybir.AluOpType

    ndp, nsp = cfg["NDp"], cfg["NSp"]
    TD, TS = cfg["TD"], cfg["TS"]
    RD, RS = TD * P, TS * P
    RLOC = RD + RS
    sched = cfg["sched"]
    idx_cols = cfg["idx_cols"]
    ds_cols = cfg["ds_cols"]
    src_off = cfg["src_off"]
    dst_off = cfg["dst_off"]
    npad_m = cfg["npad_m"]
    nreal_m = cfg["nreal_m"]
    CHMAX = max(int(rs["CHG"][i].sum()) for rs in sched
                for i in range(len(rs["CHG"])))

    nc = bacc.Bacc("TRN2", target_bir_lowering=False, debug=False,
                   num_devices=NCORES)

    ein = {}

    def EIn(name, shape, dt):
        ein[name] = nc.dram_tensor(name, shape, dt, kind="ExternalInput").ap()
        return ein[name]

    xpd = EIn("xpd", [RD, IN], f32)
    xps = EIn("xps", [RS, IN], f32)
    idx0_d = EIn("idx0", [P, max(idx_cols[0], 8)], i16)
    idx1_d = EIn("idx1", [P, max(idx_cols[1], 8)], i16)
    idxa_d = EIn("idxa", [P, max(idx_cols[2], 8)], i16)
    dslot_d = EIn("dslot", [P, ds_cols], f32)
    for nm, sh, dt in (("wp16d", [C, C], f16), ("wp16s", [C, C], f16),
                       ("bp16d", [1, C], f16), ("bp16s", [1, C], f16),
                       ("wf32d", [C, 16 + OUT], f32),
                       ("wf32s", [C, 16 + OUT], f32),
                       ("bf32d", [1, 16 + OUT], f32),
                       ("bf32s", [1, 16 + OUT], f32),
                       ("wk16", [C, C], f16), ("bk16", [1, C], f16),
                       ("q32", [1, C], f32),
                       ("wd16", [C, OUT], f16), ("bd16", [1, OUT], f16),
                       ("wdis16", [C, OUT], f16), ("bdis16", [1, OUT], f16)):
        EIn(nm, sh, dt)
    demb = nc.dram_tensor("demb", [RD, OUT], f32, kind="ExternalOutput").ap()
    semb = nc.dram_tensor("semb", [RS, OUT], f32, kind="ExternalOutput").ap()

    with tile.TileContext(nc) as tc:
        with tc.tile_pool(name="dram", bufs=1, space="DRAM") as dram, \
             tc.tile_pool(name="const", bufs=1) as cpool, \
             tc.tile_pool(name="pers", bufs=1) as pers, \
             tc.tile_pool(name="wrk", bufs=2) as wrk, \
             tc.tile_pool(name="gath", bufs=2) as gpool, \
             tc.tile_pool(name="ps_agg", bufs=2, space="PSUM") as ps_agg, \
             tc.tile_pool(name="ps_t", bufs=2, space="PSUM") as ps_t, \
             tc.tile_pool(name="ps_mm", bufs=3, space="PSUM") as ps_mm:

            floc_d = dram.tile([RD, ROW], f16)
            floc_s = dram.tile([RS, ROW], f16)
            fg_d = dram.tile([ndp, ROW], f16, addr_space="Shared")
            fg_s = dram.tile([nsp, ROW], f16, addr_space="Shared")
            aloc = dram.tile([RLOC, AROW], f32)
            cs_loc = dram.tile([1, 4 * C], f32)
            cs_glob = dram.tile([1, 4 * C], f32, addr_space="Shared")
            attn_dram = dram.tile([1, 4], f32)

            id32 = cpool.tile([P, P], f32)
            make_identity(nc, id32[:])
            id16 = cpool.tile([P, P], f16)
            make_identity(nc, id16[:])
            ones16r = cpool.tile([1, P], f16)
            nc.vector.memset(ones16r[:], 1.0)
            ones32r = cpool.tile([1, P], f32)
            nc.vector.memset(ones32r[:], 1.0)
            ones16c = cpool.tile([P, 1], f16)
            nc.vector.memset(ones16c[:], 1.0)
            iota_i = cpool.tile([P, P], i32)
            nc.gpsimd.iota(iota_i[:], pattern=[[1, P]], base=0,
                           channel_multiplier=0)
            iota32 = cpool.tile([P, P], f32)
            nc.vector.tensor_copy(iota32[:], iota_i[:])

            W = {}
            for nm, ncol in (("wp16d", C), ("wp16s", C), ("wk16", C)):
                W[nm] = cpool.tile([P, 2, ncol], f16, name=nm + "_sb")
                for k in range(2):
                    nc.sync.dma_start(W[nm][:, k, :],
                                      ein[nm][k * P:(k + 1) * P, :])
            for nm in ("wf32d", "wf32s"):
                W[nm] = cpool.tile([P, 2, 16 + OUT], f32, name=nm + "_sb")
                for k in range(2):
                    nc.sync.dma_start(W[nm][:, k, :],
                                      ein[nm][k * P:(k + 1) * P, :])
            for nm in ("wd16", "wdis16"):
                W[nm] = cpool.tile([P, 2, OUT], f16, name=nm + "_sb")
                for k in range(2):
                    nc.sync.dma_start(W[nm][:, k, :],
                                      ein[nm][k * P:(k + 1) * P, :])
            for nm in ("bp16d", "bp16s", "bk16", "bd16", "bdis16"):
                W[nm] = cpool.tile([1, ein[nm].shape[1]], f16, name=nm + "_sb")
                nc.sync.dma_start(W[nm][:], ein[nm])
            for nm in ("bf32d", "bf32s", "q32"):
                W[nm] = cpool.tile([1, ein[nm].shape[1]], f32, name=nm + "_sb")
                nc.sync.dma_start(W[nm][:], ein[nm])

            dslot_sb = pers.tile([P, ds_cols], f32)
            nc.sync.dma_start(dslot_sb[:], dslot_d)

            o_sb = pers.tile([P, (2 * TD + 2 * TS) * C], f16)
            res_sb = pers.tile([P, (TD + TS) * OUT], f16)
            cs_acc = pers.tile([1, 4 * C], f32)
            nc.vector.memset(cs_acc[:], 0.0)

            # =========== PHASE A ===========
            def phase_a(x_in, T, ty, floc, arow0, res0):
                wp, bp = W["wp16" + ty], W["bp16" + ty]
                wf, bf = W["wf32" + ty], W["bf32" + ty]
                for t in range(T):
                    xt = wrk.tile([P, IN], f32, tag="xt")
                    nc.sync.dma_start(xt[:], x_in[t * P:(t + 1) * P, :])
                    xT32 = wrk.tile([P, 2, P], f32, tag="xT32")
                    for k in range(2):
                        pt = ps_t.tile([P, P], f32, tag="pt")
                        nc.tensor.transpose(pt[:], xt[:, k * P:(k + 1) * P],
                                            id32[:])
                        nc.scalar.activation(xT32[:, k, :], pt[:], AF.Copy)
                    xT16 = wrk.tile([P, 2, P], f16, tag="xT16")
                    nc.vector.tensor_copy(
                        xT16[:].rearrange("p a b -> p (a b)"),
                        xT32[:].rearrange("p a b -> p (a b)"))
                    pj = ps_mm.tile([P, C], f32, tag="mm")
                    for k in range(2):
                        nc.tensor.matmul(pj[:], lhsT=xT16[:, k, :], rhs=wp[:, k, :],
                                         start=(k == 0), stop=False)
                    nc.tensor.matmul(pj[:], lhsT=ones16r[:], rhs=bp[:],
                                     start=False, stop=True)
                    hd16 = wrk.tile([P, C], f16, tag="hd16")
                    nc.scalar.activation(hd16[:], pj[:], AF.Copy)
                    nc.sync.dma_start(floc[t * P:(t + 1) * P, 0:C], hd16[:])
                    pf = ps_mm.tile([P, 16 + OUT], f32, tag="mm")
                    for k in range(2):
                        nc.tensor.matmul(pf[:], lhsT=xT32[:, k, :], rhs=wf[:, k, :],
                                         start=(k == 0), stop=False)
                    nc.tensor.matmul(pf[:], lhsT=ones32r[:], rhs=bf[:],
                                     start=False, stop=True)
                    asb = wrk.tile([P, 16], f32, tag="asb")
                    nc.vector.tensor_copy(asb[:], pf[:, 0:16])
                    nc.sync.dma_start(
                        floc[t * P:(t + 1) * P, C:C + 32].bitcast(f32),
                        asb[:])
                    nc.sync.dma_start(
                        aloc[arow0 + t * P:arow0 + (t + 1) * P, 0:16], asb[:])
                    nc.scalar.activation(
                        res_sb[:, (res0 + t) * OUT:(res0 + t + 1) * OUT],
                        pf[:, 16:16 + OUT], AF.Relu)

            phase_a(xpd, TD, "d", floc_d, 0, 0)
            phase_a(xps, TS, "s", floc_s, RD, TD)

            rg = [list(range(NCORES))]
            nc.gpsimd.collective_compute(
                "AllGather", ALU.bypass, replica_groups=rg,
                ins=[floc_d.opt()], outs=[fg_d.opt()])
            nc.gpsimd.collective_compute(
                "AllGather", ALU.bypass, replica_groups=rg,
                ins=[floc_s.opt()], outs=[fg_s.opt()])

            # =========== PHASE B ===========
            o_base = {"rev": 0, "dd": TD, "treats": 2 * TD, "ss": 2 * TD + TS}
            tabs = {"drug": fg_d, "dis": fg_s}
            off_i = [0, 0, 0]
            doff = 0
            for ri, rs in enumerate(sched):
                rname, stype = rs["name"], rs["stype"]
                groups, CHG, batches = rs["groups"], rs["CHG"], rs["batches"]
                ngrp = len(groups)
                so4 = src_off[rname]
                do4 = dst_off[rname]
                tab = tabs[stype]
                ob = o_base[rname]
                for b in batches:
                    nb = [int(CHG[b, g].sum()) for g in range(ngrp)]
                    tot = sum(nb)
                    featg = []
                    for g in range(ngrp):
                        if nb[g] == 0:
                            featg.append(None)
                            continue
                        idt = wrk.tile([P, max(nb[g] * 8, 8)], i16,
                                       tag=f"idxf{g}")
                        src_d = (idx0_d, idx1_d)[g]
                        nc.sync.dma_start(
                            idt[:, 0:nb[g] * 8],
                            src_d[:, off_i[g]:off_i[g] + nb[g] * 8])
                        ft = gpool.tile([P, MAXCH_G, 288], f16,
                                        tag=f"featg{g}")
                        gb, gn = groups[g]
                        _dma_gather_raw(
                            nc, ft[:, 0:nb[g], :],
                            tab[gb:gb + gn, 0:288], idt[:, 0:nb[g] * 8],
                            nb[g] * P, elem_size=288, elem_step=ROW)
                        featg.append(ft)
                        off_i[g] += nb[g] * 8
                    ida = wrk.tile([P, max(tot * 8, 8)], i16, tag="idxa")
                    nc.sync.dma_start(
                        ida[:, 0:tot * 8],
                        idxa_d[:, off_i[2]:off_i[2] + tot * 8])
                    ag = gpool.tile([P, 2 * MAXCH_G, 4], f32, tag="ag")
                    _dma_gather_raw(
                        nc, ag[:, 0:tot, :],
                        aloc[0:RLOC, do4:do4 + 4], ida[:, 0:tot * 8],
                        tot * P, elem_size=4, elem_step=AROW)
                    off_i[2] += tot * 8

                    go = [0] * ngrp
                    to = 0
                    for i in b:
                        chs = [int(CHG[i, g]) for g in range(ngrp)]
                        cht = sum(chs)
                        psum = ps_agg.tile([P, 260], f32, tag="agg")
                        kk = 0
                        for g in range(ngrp):
                            cg = chs[g]
                            if cg == 0:
                                continue
                            ft = featg[g]
                            fslice = ft[:, go[g]:go[g] + cg, :]
                            asrc = fslice[:, :, C + 2 * so4:
                                          C + 2 * so4 + 8].bitcast(f32)
                            asum = wrk.tile([P, CHMAX * 4], f32, tag="asum")
                            nc.vector.tensor_add(
                                asum[:, 0:cg * 4].rearrange(
                                    "p (k h) -> p k h", h=4),
                                asrc, ag[:, to:to + cg, :])
                            alr = wrk.tile([P, CHMAX * 4], f32, tag="alr")
                            nc.vector.tensor_scalar_mul(
                                alr[:, 0:cg * 4], asum[:, 0:cg * 4], NEG)
                            nc.vector.tensor_max(
                                alr[:, 0:cg * 4], alr[:, 0:cg * 4],
                                asum[:, 0:cg * 4])
                            expa = wrk.tile([P, CHMAX * 4], f16, tag="expa")
                            nc.scalar.activation(
                                expa[:, 0:cg * 4], alr[:, 0:cg * 4], AF.Exp)
                            oh = wrk.tile([P, CHMAX, P], f16, tag="oh")
                            nc.vector.tensor_tensor(
                                out=oh[:, 0:cg, :],
                                in0=_bc(dslot_sb[:, doff + to:doff + to + cg],
                                        P),
                                in1=_mid_bc(iota32[:], cg),
                                op=ALU.is_equal)
                            msg = wrk.tile([P, CHMAX, 260], f16, tag="msg")
                            nc.vector.tensor_tensor(
                                out=msg[:, 0:cg, 0:C].rearrange(
                                    "p k (h d) -> p k h d", h=H),
                                in0=fslice[:, :, 0:C].rearrange(
                                    "p k (h d) -> p k h d", h=H),
                                in1=_bc(expa[:, 0:cg * 4].rearrange(
                                    "p (k h) -> p k h", h=4), Dh),
                                op=ALU.mult)
                            nc.vector.tensor_copy(
                                msg[:, 0:cg, C:C + 4],
                                expa[:, 0:cg * 4].rearrange(
                                    "p (k h) -> p k h", h=4))
                            for j in range(cg):
                                nc.tensor.matmul(
                                    psum[:], lhsT=oh[:, j, :],
                                    rhs=msg[:, j, :],
                                    start=(kk == 0), stop=(kk == cht - 1))
                                kk += 1
                            go[g] += cg
                            to += cg
                        den = wrk.tile([P, 4], f32, tag="den")
                        nc.vector.tensor_scalar_add(
                            den[:], psum[:, C:C + 4], 1e-16)
                        rec = wrk.tile([P, 4], f32, tag="rec")
                        nc.vector.reciprocal(rec[:], den[:])
                        num = wrk.tile([P, C], f16, tag="num")
                        nc.scalar.activation(num[:], psum[:, 0:C], AF.Relu)
                        o_sl = o_sb[:, (ob + i) * C:(ob + i + 1) * C]
                        nc.vector.tensor_tensor(
                            out=o_sl.rearrange("p (h d) -> p h d", h=H),
                            in0=num[:].rearrange("p (h d) -> p h d", h=H),
                            in1=_bc(rec[:], Dh), op=ALU.mult)

                        oT = wrk.tile([P, 2, P], f16, tag="oT")
                        for k in range(2):
                            pt = ps_t.tile([P, P], f16, tag="pt")
                            nc.tensor.transpose(
                                pt[:], o_sl[:, k * P:(k + 1) * P], id16[:])
                            nc.scalar.activation(oT[:, k, :], pt[:], AF.Copy)
                        zp = ps_mm.tile([P, C], f32, tag="mm")
                        for k in range(2):
                            nc.tensor.matmul(zp[:], lhsT=oT[:, k, :],
                                             rhs=W["wk16"][:, k, :],
                                             start=(k == 0), stop=False)
                        nc.tensor.matmul(zp[:], lhsT=ones16r[:],
                                         rhs=W["bk16"][:],
                                         start=False, stop=True)
                        zt = wrk.tile([P, C], f16, tag="zt")
                        nc.scalar.activation(zt[:], zp[:], AF.Tanh)
                        csp = ps_mm.tile([1, C], f32, tag="mm")
                        nc.tensor.matmul(csp[:], lhsT=ones16c[:], rhs=zt[:],
                                         start=True, stop=True)
                        nc.vector.tensor_add(cs_acc[:, ri * C:(ri + 1) * C],
                                             cs_acc[:, ri * C:(ri + 1) * C],
                                             csp[:])
                    doff += tot

            # =========== AllReduce colsums; attention ===========
            nc.sync.dma_start(cs_loc[:], cs_acc[:])
            nc.gpsimd.collective_compute(
                "AllReduce", ALU.add, replica_groups=rg,
                ins=[cs_loc.opt()], outs=[cs_glob.opt()])
            cs_sb = wrk.tile([1, 4 * C], f32, tag="cs_sb")
            nc.sync.dma_start(cs_sb[:], cs_glob[:])
            zc = wrk.tile([1, C], f32, tag="zc")
            nc.scalar.activation(zc[:], W["bk16"][:], AF.Tanh)
            s4 = wrk.tile([1, 4], f32, tag="s4")
            for m in range(4):
                corr = wrk.tile([1, C], f32, tag="corr")
                nc.vector.tensor_scalar_mul(corr[:], zc[:], -float(npad_m[m]))
                nc.vector.tensor_add(corr[:], corr[:],
                                     cs_sb[:, m * C:(m + 1) * C])
                nc.vector.tensor_scalar_mul(corr[:], corr[:],
                                            1.0 / float(nreal_m[m]))
                nc.vector.tensor_mul(corr[:], corr[:], W["q32"][:])
                nc.vector.reduce_sum(s4[:, m:m + 1], corr[:],
                                     axis=mybir.AxisListType.X)
            at4 = wrk.tile([1, 4], f32, tag="at4")
            for m0 in (0, 2):
                ep = wrk.tile([1, 2], f32, tag="ep")
                nc.scalar.activation(ep[:], s4[:, m0:m0 + 2], AF.Exp)
                sd = wrk.tile([1, 1], f32, tag="sd")
                nc.vector.reduce_sum(sd[:], ep[:], axis=mybir.AxisListType.X)
                rd_ = wrk.tile([1, 1], f32, tag="rd_")
                nc.vector.reciprocal(rd_[:], sd[:])
                nc.vector.tensor_tensor(out=at4[:, m0:m0 + 2], in0=ep[:],
                                        in1=rd_[:, 0:1].to_broadcast((1, 2)),
                                        op=ALU.mult)
            nc.sync.dma_start(attn_dram[:], at4[:])
            attn_sb = wrk.tile([P, 4], f32, tag="attn_sb")
            nc.sync.dma_start(attn_sb[:], attn_dram[:].to_broadcast((P, 4)))

            # =========== PHASE C ===========
            def phase_c(T, m0, wnm, bnm, res0, out_dram):
                b0 = o_base[sched[m0]["name"]]
                b1 = o_base[sched[m0 + 1]["name"]]
                for t in range(T):
                    o0 = o_sb[:, (b0 + t) * C:(b0 + t + 1) * C]
                    o1 = o_sb[:, (b1 + t) * C:(b1 + t + 1) * C]
                    hc = wrk.tile([P, C], f16, tag="hc")
                    nc.vector.tensor_scalar(
                        out=hc[:], in0=o0, scalar1=attn_sb[:, m0:m0 + 1],
                        scalar2=None, op0=ALU.mult)
                    hc2 = wrk.tile([P, C], f16, tag="hc2")
                    nc.vector.tensor_scalar(
                        out=hc2[:], in0=o1,
                        scalar1=attn_sb[:, m0 + 1:m0 + 2],
                        scalar2=None, op0=ALU.mult)
                    nc.vector.tensor_add(hc[:], hc[:], hc2[:])
                    hT = wrk.tile([P, 2, P], f16, tag="hT")
                    for k in range(2):
                        pt = ps_t.tile([P, P], f16, tag="pt")
                        nc.tensor.transpose(pt[:], hc[:, k * P:(k + 1) * P],
                                            id16[:])
                        nc.scalar.activation(hT[:, k, :], pt[:], AF.Relu)
                    epm = ps_mm.tile([P, OUT], f32, tag="mm")
                    for k in range(2):
                        nc.tensor.matmul(epm[:], lhsT=hT[:, k, :],
                                         rhs=W[wnm][:, k, :],
                                         start=(k == 0), stop=False)
                    nc.tensor.matmul(epm[:], lhsT=ones16r[:], rhs=W[bnm][:],
                                     start=False, stop=True)
                    eo = wrk.tile([P, OUT], f32, tag="eo")
                    nc.vector.tensor_add(
                        eo[:], epm[:],
                        res_sb[:, (res0 + t) * OUT:(res0 + t + 1) * OUT])
                    nc.sync.dma_start(out_dram[t * P:(t + 1) * P, :], eo[:])

            phase_c(TD, 0, "wd16", "bd16", 0, demb)
            phase_c(TS, 2, "wdis16", "bdis16", TD, semb)

    nc.compile()
    return nc


# ---------------------------------------------------------------------------
# entry point
# ---------------------------------------------------------------------------

def _make_cfg():
    return {"ND": ND, "NS": NS, "NDp": 40960, "NSp": 30720,
            "TD": 40, "TS": 30, "SUBD": MAXSUB}


def _run(inputs, cfg):
    global _COMPILED, LAST_RESULT
    import concourse.bass_utils as bass_utils

    p = _prep(inputs, cfg)
    cfg = dict(cfg)
    cfg["sched"] = p.sched
    cfg["idx_cols"] = p.idx_cols
    cfg["ds_cols"] = p.ds_cols
    cfg["src_off"] = p.src_off
    cfg["dst_off"] = p.dst_off
    cfg["npad_m"] = [cfg["NDp"] - cfg["ND"], cfg["NDp"] - cfg["ND"],
                     cfg["NSp"] - cfg["NS"], cfg["NSp"] - cfg["NS"]]
    cfg["nreal_m"] = [cfg["ND"], cfg["ND"], cfg["NS"], cfg["NS"]]

    key = (tuple(tuple(map(tuple, rs["CHG"])) for rs in p.sched),
           tuple(p.idx_cols), p.ds_cols,
           cfg["NDp"], cfg["NSp"])
    if _COMPILED is None or _COMPILED[0] != key:
        _COMPILED = (key, _build(cfg))
    nc = _COMPILED[1]

    in_maps = []
    for c in range(NCORES):
        m = {"xpd": p.xpd[c], "xps": p.xps[c],
             "idx0": p.idx0[c], "idx1": p.idx1[c],
             "idxa": p.idxa[c], "dslot": p.dslot[c]}
        m.update(p.w)
        in_maps.append(m)

    res = bass_utils.run_bass_kernel_spmd(nc, in_maps,
                                          core_ids=list(range(NCORES)))
    LAST_RESULT = res

    demb_all = np.concatenate([res.results[c]["demb"] for c in range(NCORES)])
    semb_all = np.concatenate([res.results[c]["semb"] for c in range(NCORES)])
    drug_emb = np.zeros((cfg["ND"], OUT), np.float32)
    dis_emb = np.zeros((cfg["NS"], OUT), np.float32)
    md = p.perm_d >= 0
    drug_emb[p.perm_d[md]] = demb_all[md]
    ms = p.perm_s >= 0
    dis_emb[p.perm_s[ms]] = semb_all[ms]
    return drug_emb, dis_emb


def kernel(**inputs):
    return _run(inputs, _make_cfg())
